# revision 7
# baseline (speedup 1.0000x reference)
"""Trainium2 Bass kernel for PolyIoULoss (rotated-box IoU loss, mean reduction).

Algorithm (sort-free): work in the pred box's local frame, where pred is the
axis-aligned box |x|<=a, |y|<=b.  The intersection area of two convex polygons
equals the shoelace line integral over the clipped boundary: each of the 8
edges (4 per box), clipped against the other box's two slabs, contributes
0.5*dt*cross(start, edge_vec).  For pred edges that collapses to a*b*dt; for
target edges to dt*(a2*b2 +/- cross(delta, axis) terms).  Slab clipping is
interval arithmetic with a safe reciprocal d/(d^2+eps) so there are no NaN/Inf
paths.  Everything is branch-free elementwise math over the pair axis.

Engines: DVE (tensor_tensor incl. min/max, scalar_tensor_tensor), GPSIMD
(tensor_tensor add/sub/mult + tensor_scalar only), ACT (Sin/Ln/Exp/Square/
Abs/Relu + free affine, accum_out for the final sum).  1/x is exp(-ln(x)).

Sharding: embarrassingly data-parallel - 524288 pairs split as 65536 per core
(8 cores), laid out [128 partitions x 512 free].  Each core returns per-
partition sums of log(iou); the host combines and negates for the mean.
"""

import numpy as np

import concourse.bacc as bacc
import concourse.tile as tile
from concourse import mybir
from concourse.mybir import AluOpType as Op, ActivationFunctionType as Fn
from concourse.bass_utils import run_bass_kernel_spmd

N_TOTAL = 524288
NCORES = 8
NPER = N_TOTAL // NCORES          # 65536 pairs per core
P = 128
F = NPER // P                     # 512 pairs per partition

HPI = float(np.pi / 2)
EPS = 1e-6                        # from the loss definition
REPS = 1e-20                      # safe-reciprocal regularizer
F32 = mybir.dt.float32


class _Var:
    __slots__ = ("ap", "tag")

    def __init__(self, ap, tag):
        self.ap = ap
        self.tag = tag


def _ap(x):
    return x.ap[:, :] if isinstance(x, _Var) else x


class _Slots:
    """Manual slot allocator over Tile pool tags: reusing a tag reuses the
    SBUF slot (Tile inserts the WAR dependency), so peak SBUF = live tags."""

    def __init__(self, pool):
        self.pool = pool
        self.free_tags = []
        self.count = 0

    def tile(self):
        if self.free_tags:
            tag = self.free_tags.pop()
        else:
            tag = f"s{self.count}"
            self.count += 1
        t = self.pool.tile([P, F], F32, tag=tag)
        return _Var(t, tag)

    def free(self, *tiles):
        for t in tiles:
            self.free_tags.append(t.tag)


def _build():
    nc = bacc.Bacc(None, target_bir_lowering=False)
    pred_h = nc.dram_tensor("pred", [NPER, 5], F32, kind="ExternalInput")
    tgt_h = nc.dram_tensor("target", [NPER, 5], F32, kind="ExternalInput")
    out_h = nc.dram_tensor("out", [P, 1], F32, kind="ExternalOutput")

    V = nc.vector
    G = nc.gpsimd
    A = nc.scalar

    with tile.TileContext(nc) as tc:
        with tc.tile_pool(name="io", bufs=1) as io_pool, \
             tc.tile_pool(name="consts", bufs=1) as cpool, \
             tc.tile_pool(name="main", bufs=1) as pool:
            s = _Slots(pool)

            _kcache = {}

            def konst(val):
                # [P,1] SBUF constant, for activation bias/scale operands
                if val not in _kcache:
                    t = cpool.tile([P, 1], F32, tag=f"c{len(_kcache)}")
                    nc.gpsimd.memset(t[:, :], val)
                    _kcache[val] = t
                return _kcache[val][:, :]

            PRED = io_pool.tile([P, F, 5], F32, tag="PRED")
            TGT = io_pool.tile([P, F, 5], F32, tag="TGT")
            nc.sync.dma_start(out=PRED, in_=pred_h[:, :].rearrange("(p f) c -> p f c", p=P))
            nc.sync.dma_start(out=TGT, in_=tgt_h[:, :].rearrange("(p f) c -> p f c", p=P))

            pcx, pcy, pw, ph, pth = (PRED[:, :, j] for j in range(5))
            tcx, tcy, tw, th, tth = (TGT[:, :, j] for j in range(5))

            def tt(eng, x, y, op):
                o = s.tile()
                eng.tensor_tensor(_ap(o), _ap(x), _ap(y), op)
                return o

            def ts(eng, x, scalar, op):
                o = s.tile()
                eng.tensor_scalar(_ap(o), _ap(x), scalar, None, op)
                return o

            def stt(x, scalar, y, op0, op1):
                # DVE only: (x op0 scalar) op1 y
                o = s.tile()
                V.scalar_tensor_tensor(_ap(o), _ap(x), scalar, _ap(y), op0, op1)
                return o

            def act(x, func, bias=0.0, scale=1.0, accum_out=None):
                o = s.tile()
                if isinstance(bias, float) and bias not in (0.0, 1.0):
                    bias = konst(bias)
                if isinstance(scale, float) and scale not in (0.0, 1.0):
                    scale = konst(scale)
                A.activation(out=_ap(o), in_=_ap(x),
                             func=func, bias=bias, scale=scale, accum_out=accum_out)
                return o

            # ---- Phase A: angles, sizes, deltas --------------------------------
            phi = tt(V, tth, pth, Op.subtract)
            cosp = act(pth, Fn.Sin, bias=HPI)
            sinp = act(pth, Fn.Sin)
            cosf = act(phi, Fn.Sin, bias=HPI)
            sinf = act(phi, Fn.Sin)
            s.free(phi)
            a = act(pw, Fn.Copy, scale=0.5)
            b = act(ph, Fn.Copy, scale=0.5)
            a2 = act(tw, Fn.Copy, scale=0.5)
            b2 = act(th, Fn.Copy, scale=0.5)
            ar1 = tt(V, pw, ph, Op.mult)
            ar2 = tt(G, tw, th, Op.mult)
            Dx = tt(V, tcx, pcx, Op.subtract)
            Dy = tt(V, tcy, pcy, Op.subtract)

            t1 = tt(V, Dx, cosp, Op.mult)
            t2 = tt(V, Dy, sinp, Op.mult)
            dx = tt(V, t1, t2, Op.subtract)
            s.free(t1, t2)
            t3 = tt(G, Dx, sinp, Op.mult)
            t4 = tt(G, Dy, cosp, Op.mult)
            dy = tt(G, t3, t4, Op.add)
            s.free(t3, t4, Dx, Dy, cosp, sinp)

            ab = tt(V, a, b, Op.mult)
            ab2 = tt(G, a2, b2, Op.mult)

            # ---- Phase B: target geometry in pred frame ------------------------
            A_ = tt(V, a2, cosf, Op.mult)
            B_ = tt(V, b2, sinf, Op.mult)
            C_ = tt(G, a2, sinf, Op.mult)
            D_ = tt(G, b2, cosf, Op.mult)

            m1 = tt(V, dx, C_, Op.mult)
            m2 = tt(V, dy, A_, Op.mult)
            uu = tt(V, m1, m2, Op.add)
            s.free(m1, m2)
            m3 = tt(G, dx, D_, Op.mult)
            m4 = tt(G, dy, B_, Op.mult)
            vv = tt(G, m3, m4, Op.subtract)
            s.free(m3, m4)

            S1 = tt(V, A_, B_, Op.add)
            S2 = tt(V, A_, B_, Op.subtract)
            S3 = tt(G, C_, D_, Op.add)
            S4 = tt(G, C_, D_, Op.subtract)
            g0x = tt(V, dx, S1, Op.subtract)
            g2x = tt(V, dx, S1, Op.add)
            g1x = tt(V, dx, S2, Op.add)
            g3x = tt(V, dx, S2, Op.subtract)
            s.free(S1, S2)
            g0y = tt(G, dy, S4, Op.add)
            g2y = tt(G, dy, S4, Op.subtract)
            g1y = tt(G, dy, S3, Op.subtract)
            g3y = tt(G, dy, S3, Op.add)
            s.free(S3, S4)

            # ---- Phase C: pred corners in target frame -------------------------
            e1 = tt(V, a, dx, Op.subtract)
            e2 = stt(a, -1.0, dx, Op.mult, Op.subtract)     # -a - dx
            f1 = tt(G, b, dy, Op.subtract)
            f2 = stt(b, -1.0, dy, Op.mult, Op.subtract)     # -b - dy
            s.free(dx, dy)

            e1c = tt(V, e1, cosf, Op.mult)
            e2c = tt(V, e2, cosf, Op.mult)
            f1s = tt(V, f1, sinf, Op.mult)
            f2s = tt(V, f2, sinf, Op.mult)
            e1s = tt(G, e1, sinf, Op.mult)
            e2s = tt(G, e2, sinf, Op.mult)
            f1c = tt(G, f1, cosf, Op.mult)
            f2c = tt(G, f2, cosf, Op.mult)
            s.free(e1, e2, f1, f2, cosf, sinf)

            xi0 = tt(V, e2c, f2s, Op.subtract)
            xi1 = tt(V, e1c, f2s, Op.subtract)
            xi2 = tt(V, e1c, f1s, Op.subtract)
            xi3 = tt(V, e2c, f1s, Op.subtract)
            et0 = tt(G, e2s, f2c, Op.add)
            et1 = tt(G, e1s, f2c, Op.add)
            et2 = tt(G, e1s, f1c, Op.add)
            et3 = tt(G, e2s, f1c, Op.add)
            u1 = tt(V, e1c, e2c, Op.subtract)
            w1 = tt(V, f2s, f1s, Op.subtract)
            u2 = tt(G, e1s, e2s, Op.subtract)
            w2 = tt(G, f1c, f2c, Op.subtract)
            s.free(e1c, e2c, f1s, f2s, e1s, e2s, f1c, f2c)

            # ---- Phase D: safe reciprocals of the 8 edge slopes ----------------
            def sinv(eng, x, scale):
                # safe reciprocal of (scale*x): (scale*x) / ((scale*x)^2 + REPS)
                # 1/u computed as exp(-ln(u)) on ACT (u > 0 always).
                q = act(x, Fn.Square, scale=scale)
                l = act(q, Fn.Ln, bias=REPS)
                s.free(q)
                r = act(l, Fn.Exp, scale=-1.0)
                s.free(l)
                if scale == 1.0:
                    o = tt(eng, x, r, Op.mult)
                else:
                    o = stt(x, scale, r, Op.mult, Op.mult)
                s.free(r)
                return o

            iu1 = sinv(G, u1, 1.0)
            iu2 = sinv(G, u2, 1.0)
            iw1 = sinv(G, w1, 1.0)
            iw2 = sinv(G, w2, 1.0)
            iA = sinv(V, A_, 2.0)
            iC = sinv(V, C_, 2.0)
            iB = sinv(V, B_, 2.0)
            iD = sinv(V, D_, 2.0)
            s.free(u1, u2, w1, w2, A_, B_, C_, D_)

            # ---- Phase E: slab alphas |bound * inv| ----------------------------
            def alpha(eng, m, inv):
                t = tt(eng, m, inv, Op.mult)
                o = act(t, Fn.Abs)
                s.free(t)
                return o

            AXU = alpha(V, a2, iu1)
            AEU = alpha(G, b2, iu2)
            AXW = alpha(V, a2, iw1)
            AEW = alpha(G, b2, iw2)
            AXA = alpha(V, a, iA)
            AYC = alpha(G, b, iC)
            AXB = alpha(V, a, iB)
            AYD = alpha(G, b, iD)
            s.free(a, b, a2, b2)

            # ---- Phase F: per-edge clip intervals ------------------------------
            def edge(eng, c1, i1, s1, Aa1, c2, i2, s2, Aa2):
                # eng: engine for add/sub/mult ops (min ops + TS stay on DVE)
                b1 = tt(eng, c1, i1, Op.mult)
                b2_ = tt(eng, c2, i2, Op.mult)
                # n = -tlo ; h = thi   (sign depends on slope orientation)
                n1 = tt(eng, b1, Aa1, Op.add) if s1 > 0 else tt(eng, Aa1, b1, Op.subtract)
                h1 = tt(eng, Aa1, b1, Op.subtract) if s1 > 0 else tt(eng, Aa1, b1, Op.add)
                n2 = tt(eng, b2_, Aa2, Op.add) if s2 > 0 else tt(eng, Aa2, b2_, Op.subtract)
                h2 = tt(eng, Aa2, b2_, Op.subtract) if s2 > 0 else tt(eng, Aa2, b2_, Op.add)
                s.free(b1, b2_)
                nlo = tt(V, n1, n2, Op.min)      # = -lo
                hi = tt(V, h1, h2, Op.min)
                s.free(n1, n2, h1, h2)
                hi1 = ts(V, hi, 1.0, Op.min)
                rlo = act(nlo, Fn.Relu, scale=-1.0)  # relu(lo)
                s.free(nlo, hi)
                dte = tt(eng, hi1, rlo, Op.subtract)
                s.free(hi1, rlo)
                dt = act(dte, Fn.Relu)
                s.free(dte)
                return dt

            dtP0 = edge(V, xi0, iu1, +1, AXU, et0, iu2, +1, AEU)
            dtP1 = edge(G, xi1, iw1, +1, AXW, et1, iw2, +1, AEW)
            dtP2 = edge(G, xi2, iu1, -1, AXU, et2, iu2, -1, AEU)
            dtP3 = edge(V, xi3, iw1, -1, AXW, et3, iw2, -1, AEW)
            dtT0 = edge(G, g0x, iA, +1, AXA, g0y, iC, -1, AYC)
            dtT1 = edge(V, g1x, iB, +1, AXB, g1y, iD, +1, AYD)
            dtT2 = edge(G, g2x, iA, -1, AXA, g2y, iC, +1, AYC)
            dtT3 = edge(V, g3x, iB, -1, AXB, g3y, iD, -1, AYD)
            s.free(xi0, xi1, xi2, xi3, et0, et1, et2, et3,
                   g0x, g1x, g2x, g3x, g0y, g1y, g2y, g3y,
                   iu1, iu2, iw1, iw2, iA, iB, iC, iD,
                   AXU, AEU, AXW, AEW, AXA, AYC, AXB, AYD)

            # ---- Phase G: overlap, iou, loss -----------------------------------
            sp1 = tt(V, dtP0, dtP1, Op.add)
            sp2 = tt(G, dtP2, dtP3, Op.add)
            sp = tt(V, sp1, sp2, Op.add)
            ovP = tt(V, ab, sp, Op.mult)
            s.free(sp1, sp2, sp, ab, dtP0, dtP1, dtP2, dtP3)

            x0 = tt(V, ab2, uu, Op.subtract)
            c0 = tt(V, dtT0, x0, Op.mult)
            x2 = tt(V, ab2, uu, Op.add)
            c2_ = tt(V, dtT2, x2, Op.mult)
            x1 = tt(G, ab2, vv, Op.add)
            c1_ = tt(G, dtT1, x1, Op.mult)
            x3 = tt(G, ab2, vv, Op.subtract)
            c3_ = tt(G, dtT3, x3, Op.mult)
            s.free(x0, x1, x2, x3, uu, vv, ab2, dtT0, dtT1, dtT2, dtT3)
            cs1 = tt(V, c0, c2_, Op.add)
            cs2 = tt(G, c1_, c3_, Op.add)
            ovT = tt(V, cs1, cs2, Op.add)
            ov = tt(V, ovP, ovT, Op.add)
            s.free(c0, c1_, c2_, c3_, cs1, cs2, ovP, ovT)

            s12 = tt(G, ar1, ar2, Op.add)
            den = tt(V, s12, ov, Op.subtract)
            s.free(ar1, ar2, s12)
            dl = act(den, Fn.Ln, bias=EPS)
            s.free(den)
            rd = act(dl, Fn.Exp, scale=-1.0)
            s.free(dl)
            iour = tt(V, ov, rd, Op.mult)
            s.free(ov, rd)
            iou = ts(V, iour, EPS, Op.max)
            s.free(iour)

            acc = pool.tile([P, 1], F32, tag="acc")
            lg = act(iou, Fn.Ln, accum_out=acc[:, 0:1])
            s.free(iou, lg)

            nc.sync.dma_start(out=out_h[:, :], in_=acc[:, :])

    nc.compile()
    return nc


_NC = None


def _get_nc():
    global _NC
    if _NC is None:
        _NC = _build()
    return _NC


def kernel(pred: np.ndarray, target: np.ndarray) -> np.ndarray:
    pred = np.ascontiguousarray(np.asarray(pred, dtype=np.float32))
    target = np.ascontiguousarray(np.asarray(target, dtype=np.float32))
    assert pred.shape == (N_TOTAL, 5) and target.shape == (N_TOTAL, 5)

    nc = _get_nc()
    in_maps = [
        {
            "pred": pred[c * NPER:(c + 1) * NPER],
            "target": target[c * NPER:(c + 1) * NPER],
        }
        for c in range(NCORES)
    ]
    res = run_bass_kernel_spmd(nc, in_maps, core_ids=list(range(NCORES)))
    total = 0.0
    for r in res.results:
        total += r["out"].astype(np.float64).sum()
    return np.float32(-(total / N_TOTAL))


# revision 8
# speedup vs baseline: 1.0802x; 1.0802x over previous
"""Trainium2 Bass kernel for PolyIoULoss (rotated-box IoU loss, mean reduction).

Algorithm (sort-free): work in the pred box's local frame, where pred is the
axis-aligned box |x|<=a, |y|<=b.  The intersection area of two convex polygons
equals the shoelace line integral over the clipped boundary: each of the 8
edges (4 per box), clipped against the other box's two slabs, contributes
0.5*dt*cross(start, edge_vec).  For pred edges that collapses to a*b*dt; for
target edges to dt*(a2*b2 +/- cross(delta, axis) terms).  Slab clipping is
interval arithmetic with a safe reciprocal d/(d^2+eps) so there are no NaN/Inf
paths.  Everything is branch-free elementwise math over the pair axis.

Engines: DVE (tensor_tensor incl. min/max, scalar_tensor_tensor), GPSIMD
(tensor_tensor add/sub/mult + tensor_scalar only), ACT (Sin/Ln/Exp/Square/
Abs/Relu + free affine, accum_out for the final sum).  1/x is exp(-ln(x)).

Sharding: embarrassingly data-parallel - 524288 pairs split as 65536 per core
(8 cores), laid out [128 partitions x 512 free].  Each core returns per-
partition sums of log(iou); the host combines and negates for the mean.
"""

import numpy as np

import concourse.bacc as bacc
import concourse.tile as tile
from concourse import mybir
from concourse.mybir import AluOpType as Op, ActivationFunctionType as Fn
from concourse.bass_utils import run_bass_kernel_spmd

N_TOTAL = 524288
NCORES = 8
NPER = N_TOTAL // NCORES          # 65536 pairs per core
P = 128
F = NPER // P                     # 512 pairs per partition

HPI = float(np.pi / 2)
EPS = 1e-6                        # from the loss definition
REPS = 1e-20                      # safe-reciprocal regularizer
F32 = mybir.dt.float32


class _Var:
    __slots__ = ("ap", "tag")

    def __init__(self, ap, tag):
        self.ap = ap
        self.tag = tag


def _ap(x):
    return x.ap[:, :] if isinstance(x, _Var) else x


class _Slots:
    """Manual slot allocator over Tile pool tags: reusing a tag reuses the
    SBUF slot (Tile inserts the WAR dependency), so peak SBUF = live tags."""

    def __init__(self, pool):
        self.pool = pool
        self.free_tags = []
        self.count = 0

    def tile(self):
        if self.free_tags:
            tag = self.free_tags.pop()
        else:
            tag = f"s{self.count}"
            self.count += 1
        t = self.pool.tile([P, F], F32, tag=tag)
        return _Var(t, tag)

    def free(self, *tiles):
        for t in tiles:
            self.free_tags.append(t.tag)


def _build():
    nc = bacc.Bacc(None, target_bir_lowering=False)
    pred_h = nc.dram_tensor("pred", [NPER, 5], F32, kind="ExternalInput")
    tgt_h = nc.dram_tensor("target", [NPER, 5], F32, kind="ExternalInput")
    out_h = nc.dram_tensor("out", [P, 1], F32, kind="ExternalOutput")

    V = nc.vector
    G = nc.gpsimd
    A = nc.scalar

    with tile.TileContext(nc) as tc:
        with tc.tile_pool(name="io", bufs=1) as io_pool, \
             tc.tile_pool(name="consts", bufs=1) as cpool, \
             tc.tile_pool(name="main", bufs=1) as pool:
            s = _Slots(pool)

            _kcache = {}

            def konst(val):
                # [P,1] SBUF constant, for activation bias/scale operands
                if val not in _kcache:
                    t = cpool.tile([P, 1], F32, tag=f"c{len(_kcache)}")
                    nc.gpsimd.memset(t[:, :], val)
                    _kcache[val] = t
                return _kcache[val][:, :]

            PRED = io_pool.tile([P, F, 5], F32, tag="PRED")
            TGT = io_pool.tile([P, F, 5], F32, tag="TGT")
            nc.sync.dma_start(out=PRED, in_=pred_h[:, :].rearrange("(p f) c -> p f c", p=P))
            nc.sync.dma_start(out=TGT, in_=tgt_h[:, :].rearrange("(p f) c -> p f c", p=P))

            pcx, pcy, pw, ph, pth = (PRED[:, :, j] for j in range(5))
            tcx, tcy, tw, th, tth = (TGT[:, :, j] for j in range(5))

            def tt(eng, x, y, op):
                o = s.tile()
                eng.tensor_tensor(_ap(o), _ap(x), _ap(y), op)
                return o

            def ts(eng, x, scalar, op):
                o = s.tile()
                eng.tensor_scalar(_ap(o), _ap(x), scalar, None, op)
                return o

            def stt(x, scalar, y, op0, op1):
                # DVE only: (x op0 scalar) op1 y
                o = s.tile()
                V.scalar_tensor_tensor(_ap(o), _ap(x), scalar, _ap(y), op0, op1)
                return o

            def act(x, func, bias=0.0, scale=1.0, accum_out=None):
                o = s.tile()
                if isinstance(bias, float) and bias not in (0.0, 1.0):
                    bias = konst(bias)
                if isinstance(scale, float) and scale not in (0.0, 1.0):
                    scale = konst(scale)
                A.activation(out=_ap(o), in_=_ap(x),
                             func=func, bias=bias, scale=scale, accum_out=accum_out)
                return o

            # ---- Phase A: angles, sizes, deltas --------------------------------
            # ACT Sin is only accurate on ~(-pi, pi): cos(t) = sin(pi/2 - t)
            # works for t in [0, pi); cos(phi) for phi in (-pi, pi) uses the
            # half-angle identity 1 - 2*sin^2(phi/2).
            phi = tt(V, tth, pth, Op.subtract)
            cosp = act(pth, Fn.Sin, bias=HPI, scale=-1.0)
            sinp = act(pth, Fn.Sin)
            sinf = act(phi, Fn.Sin)
            sh = act(phi, Fn.Sin, scale=0.5)
            s.free(phi)
            shq = act(sh, Fn.Square)
            s.free(sh)
            cosf = s.tile()
            V.tensor_scalar(_ap(cosf), _ap(shq), -2.0, 1.0, Op.mult, Op.add)
            s.free(shq)
            a = act(pw, Fn.Copy, scale=0.5)
            b = act(ph, Fn.Copy, scale=0.5)
            a2 = act(tw, Fn.Copy, scale=0.5)
            b2 = act(th, Fn.Copy, scale=0.5)
            ar1 = tt(V, pw, ph, Op.mult)
            ar2 = tt(G, tw, th, Op.mult)
            Dx = tt(V, tcx, pcx, Op.subtract)
            Dy = tt(V, tcy, pcy, Op.subtract)

            t1 = tt(V, Dx, cosp, Op.mult)
            t2 = tt(V, Dy, sinp, Op.mult)
            dx = tt(V, t1, t2, Op.subtract)
            s.free(t1, t2)
            t3 = tt(G, Dx, sinp, Op.mult)
            t4 = tt(G, Dy, cosp, Op.mult)
            dy = tt(G, t3, t4, Op.add)
            s.free(t3, t4, Dx, Dy, cosp, sinp)

            ab = tt(V, a, b, Op.mult)
            ab2 = tt(G, a2, b2, Op.mult)

            # ---- Phase B: target geometry in pred frame ------------------------
            A_ = tt(V, a2, cosf, Op.mult)
            B_ = tt(V, b2, sinf, Op.mult)
            C_ = tt(G, a2, sinf, Op.mult)
            D_ = tt(G, b2, cosf, Op.mult)

            m1 = tt(V, dx, C_, Op.mult)
            m2 = tt(V, dy, A_, Op.mult)
            uu = tt(V, m1, m2, Op.add)
            s.free(m1, m2)
            m3 = tt(G, dx, D_, Op.mult)
            m4 = tt(G, dy, B_, Op.mult)
            vv = tt(G, m3, m4, Op.subtract)
            s.free(m3, m4)

            S1 = tt(V, A_, B_, Op.add)
            S2 = tt(V, A_, B_, Op.subtract)
            S3 = tt(G, C_, D_, Op.add)
            S4 = tt(G, C_, D_, Op.subtract)
            g0x = tt(V, dx, S1, Op.subtract)
            g2x = tt(V, dx, S1, Op.add)
            g1x = tt(V, dx, S2, Op.add)
            g3x = tt(V, dx, S2, Op.subtract)
            s.free(S1, S2)
            g0y = tt(G, dy, S4, Op.add)
            g2y = tt(G, dy, S4, Op.subtract)
            g1y = tt(G, dy, S3, Op.subtract)
            g3y = tt(G, dy, S3, Op.add)
            s.free(S3, S4)

            # ---- Phase C: pred corners in target frame -------------------------
            e1 = tt(V, a, dx, Op.subtract)
            e2 = stt(a, -1.0, dx, Op.mult, Op.subtract)     # -a - dx
            f1 = tt(G, b, dy, Op.subtract)
            f2 = stt(b, -1.0, dy, Op.mult, Op.subtract)     # -b - dy
            s.free(dx, dy)

            e1c = tt(V, e1, cosf, Op.mult)
            e2c = tt(V, e2, cosf, Op.mult)
            f1s = tt(V, f1, sinf, Op.mult)
            f2s = tt(V, f2, sinf, Op.mult)
            e1s = tt(G, e1, sinf, Op.mult)
            e2s = tt(G, e2, sinf, Op.mult)
            f1c = tt(G, f1, cosf, Op.mult)
            f2c = tt(G, f2, cosf, Op.mult)
            s.free(e1, e2, f1, f2, cosf, sinf)

            xi0 = tt(V, e2c, f2s, Op.subtract)
            xi1 = tt(V, e1c, f2s, Op.subtract)
            xi2 = tt(V, e1c, f1s, Op.subtract)
            xi3 = tt(V, e2c, f1s, Op.subtract)
            et0 = tt(G, e2s, f2c, Op.add)
            et1 = tt(G, e1s, f2c, Op.add)
            et2 = tt(G, e1s, f1c, Op.add)
            et3 = tt(G, e2s, f1c, Op.add)
            u1 = tt(V, e1c, e2c, Op.subtract)
            w1 = tt(V, f2s, f1s, Op.subtract)
            u2 = tt(G, e1s, e2s, Op.subtract)
            w2 = tt(G, f1c, f2c, Op.subtract)
            s.free(e1c, e2c, f1s, f2s, e1s, e2s, f1c, f2c)

            # ---- Phase D: safe reciprocals of the 8 edge slopes ----------------
            def sinv(eng, x, scale):
                # safe reciprocal of (scale*x): (scale*x) / ((scale*x)^2 + REPS)
                # 1/u computed as exp(-ln(u)) on ACT (u > 0 always).
                q = act(x, Fn.Square, scale=scale)
                l = act(q, Fn.Ln, bias=REPS)
                s.free(q)
                r = act(l, Fn.Exp, scale=-1.0)
                s.free(l)
                if scale == 1.0:
                    o = tt(eng, x, r, Op.mult)
                else:
                    o = stt(x, scale, r, Op.mult, Op.mult)
                s.free(r)
                return o

            iu1 = sinv(G, u1, 1.0)
            iu2 = sinv(G, u2, 1.0)
            iw1 = sinv(G, w1, 1.0)
            iw2 = sinv(G, w2, 1.0)
            iA = sinv(V, A_, 2.0)
            iC = sinv(V, C_, 2.0)
            iB = sinv(V, B_, 2.0)
            iD = sinv(V, D_, 2.0)
            s.free(u1, u2, w1, w2, A_, B_, C_, D_)

            # ---- Phase E: slab alphas |bound * inv| ----------------------------
            def alpha(eng, m, inv):
                t = tt(eng, m, inv, Op.mult)
                o = act(t, Fn.Abs)
                s.free(t)
                return o

            AXU = alpha(V, a2, iu1)
            AEU = alpha(G, b2, iu2)
            AXW = alpha(V, a2, iw1)
            AEW = alpha(G, b2, iw2)
            AXA = alpha(V, a, iA)
            AYC = alpha(G, b, iC)
            AXB = alpha(V, a, iB)
            AYD = alpha(G, b, iD)
            s.free(a, b, a2, b2)

            # ---- Phase F: per-edge clip intervals ------------------------------
            def edge(eng, c1, i1, s1, Aa1, c2, i2, s2, Aa2):
                # eng: engine for add/sub/mult ops (min ops + TS stay on DVE)
                b1 = tt(eng, c1, i1, Op.mult)
                b2_ = tt(eng, c2, i2, Op.mult)
                # n = -tlo ; h = thi   (sign depends on slope orientation)
                n1 = tt(eng, b1, Aa1, Op.add) if s1 > 0 else tt(eng, Aa1, b1, Op.subtract)
                h1 = tt(eng, Aa1, b1, Op.subtract) if s1 > 0 else tt(eng, Aa1, b1, Op.add)
                n2 = tt(eng, b2_, Aa2, Op.add) if s2 > 0 else tt(eng, Aa2, b2_, Op.subtract)
                h2 = tt(eng, Aa2, b2_, Op.subtract) if s2 > 0 else tt(eng, Aa2, b2_, Op.add)
                s.free(b1, b2_)
                nlo = tt(V, n1, n2, Op.min)      # = -lo
                hi = tt(V, h1, h2, Op.min)
                s.free(n1, n2, h1, h2)
                hi1 = ts(V, hi, 1.0, Op.min)
                rlo = act(nlo, Fn.Relu, scale=-1.0)  # relu(lo)
                s.free(nlo, hi)
                dte = tt(eng, hi1, rlo, Op.subtract)
                s.free(hi1, rlo)
                dt = act(dte, Fn.Relu)
                s.free(dte)
                return dt

            dtP0 = edge(V, xi0, iu1, +1, AXU, et0, iu2, +1, AEU)
            dtP1 = edge(G, xi1, iw1, +1, AXW, et1, iw2, +1, AEW)
            dtP2 = edge(G, xi2, iu1, -1, AXU, et2, iu2, -1, AEU)
            dtP3 = edge(V, xi3, iw1, -1, AXW, et3, iw2, -1, AEW)
            dtT0 = edge(G, g0x, iA, +1, AXA, g0y, iC, -1, AYC)
            dtT1 = edge(V, g1x, iB, +1, AXB, g1y, iD, +1, AYD)
            dtT2 = edge(G, g2x, iA, -1, AXA, g2y, iC, +1, AYC)
            dtT3 = edge(V, g3x, iB, -1, AXB, g3y, iD, -1, AYD)
            s.free(xi0, xi1, xi2, xi3, et0, et1, et2, et3,
                   g0x, g1x, g2x, g3x, g0y, g1y, g2y, g3y,
                   iu1, iu2, iw1, iw2, iA, iB, iC, iD,
                   AXU, AEU, AXW, AEW, AXA, AYC, AXB, AYD)

            # ---- Phase G: overlap, iou, loss -----------------------------------
            sp1 = tt(V, dtP0, dtP1, Op.add)
            sp2 = tt(G, dtP2, dtP3, Op.add)
            sp = tt(V, sp1, sp2, Op.add)
            ovP = tt(V, ab, sp, Op.mult)
            s.free(sp1, sp2, sp, ab, dtP0, dtP1, dtP2, dtP3)

            x0 = tt(V, ab2, uu, Op.subtract)
            c0 = tt(V, dtT0, x0, Op.mult)
            x2 = tt(V, ab2, uu, Op.add)
            c2_ = tt(V, dtT2, x2, Op.mult)
            x1 = tt(G, ab2, vv, Op.add)
            c1_ = tt(G, dtT1, x1, Op.mult)
            x3 = tt(G, ab2, vv, Op.subtract)
            c3_ = tt(G, dtT3, x3, Op.mult)
            s.free(x0, x1, x2, x3, uu, vv, ab2, dtT0, dtT1, dtT2, dtT3)
            cs1 = tt(V, c0, c2_, Op.add)
            cs2 = tt(G, c1_, c3_, Op.add)
            ovT = tt(V, cs1, cs2, Op.add)
            ov = tt(V, ovP, ovT, Op.add)
            s.free(c0, c1_, c2_, c3_, cs1, cs2, ovP, ovT)

            s12 = tt(G, ar1, ar2, Op.add)
            den = tt(V, s12, ov, Op.subtract)
            s.free(ar1, ar2, s12)
            dl = act(den, Fn.Ln, bias=EPS)
            s.free(den)
            rd = act(dl, Fn.Exp, scale=-1.0)
            s.free(dl)
            iour = tt(V, ov, rd, Op.mult)
            s.free(ov, rd)
            iou = ts(V, iour, EPS, Op.max)
            s.free(iour)

            acc = pool.tile([P, 1], F32, tag="acc")
            lg = act(iou, Fn.Ln, accum_out=acc[:, 0:1])
            s.free(iou, lg)

            nc.sync.dma_start(out=out_h[:, :], in_=acc[:, :])

    nc.compile()
    return nc


_NC = None


def _get_nc():
    global _NC
    if _NC is None:
        _NC = _build()
    return _NC


def kernel(pred: np.ndarray, target: np.ndarray) -> np.ndarray:
    pred = np.ascontiguousarray(np.asarray(pred, dtype=np.float32))
    target = np.ascontiguousarray(np.asarray(target, dtype=np.float32))
    assert pred.shape == (N_TOTAL, 5) and target.shape == (N_TOTAL, 5)

    nc = _get_nc()
    in_maps = [
        {
            "pred": pred[c * NPER:(c + 1) * NPER],
            "target": target[c * NPER:(c + 1) * NPER],
        }
        for c in range(NCORES)
    ]
    res = run_bass_kernel_spmd(nc, in_maps, core_ids=list(range(NCORES)))
    total = 0.0
    for r in res.results:
        total += r["out"].astype(np.float64).sum()
    return np.float32(-(total / N_TOTAL))


# revision 17
# speedup vs baseline: 4357.2753x; 4033.7930x over previous
"""Trainium2 Bass kernel for PolyIoULoss (rotated-box IoU loss, mean reduction).

Algorithm (sort-free): work in the pred box's local frame, where pred is the
axis-aligned box |x|<=a, |y|<=b.  The intersection area of two convex polygons
equals the shoelace line integral over the clipped boundary: each of the 8
edges (4 per box), clipped against the other box's two slabs, contributes
0.5*dt*cross(start, edge_vec).  For pred edges that collapses to a*b*dt; for
target edges to dt*(a2*b2 +/- cross(delta, axis) terms).  Slab clipping is
interval arithmetic with a safe reciprocal d/(d^2+eps) so there are no NaN/Inf
paths.  Everything is branch-free elementwise math over the pair axis.

Engines: DVE (tensor_tensor incl. min/max, scalar_tensor_tensor), GPSIMD
(tensor_tensor add/sub/mult + tensor_scalar only), ACT (Sin/Ln/Exp/Square/
Abs/Relu + free affine, accum_out for the final sum).  1/x is exp(-ln(x)).

Sharding: embarrassingly data-parallel - 524288 pairs split as 65536 per core
(8 cores), laid out [128 partitions x 512 free].  Each core returns per-
partition sums of log(iou); the host combines and negates for the mean.
"""

import numpy as np

import concourse.bacc as bacc
import concourse.tile as tile
from concourse import mybir
from concourse.mybir import AluOpType as Op, ActivationFunctionType as Fn
from concourse.bass_utils import run_bass_kernel_spmd

N_TOTAL = 524288
NCORES = 8
NPER = N_TOTAL // NCORES          # 65536 pairs per core
P = 128
F = NPER // P                     # 512 pairs per partition

HPI = float(np.pi / 2)
EPS = 1e-6                        # from the loss definition
REPS = 1e-20                      # safe-reciprocal regularizer
F32 = mybir.dt.float32


class _Var:
    __slots__ = ("ap", "tag")

    def __init__(self, ap, tag):
        self.ap = ap
        self.tag = tag


def _ap(x):
    return x.ap[:, :] if isinstance(x, _Var) else x


class _Slots:
    """Manual slot allocator over Tile pool tags: reusing a tag reuses the
    SBUF slot (Tile inserts the WAR dependency), so peak SBUF = distinct tags.
    FIFO reuse + slack slots maximize the distance between a free and the
    next write to the same slot, so WAR deps don't serialize the schedule."""

    MAX_TAGS = 78   # 78 * 2KB/partition + 20KB inputs + consts < 192KB SBUF

    def __init__(self, pool):
        self.pool = pool
        import collections
        self.free_tags = collections.deque()
        self.count = 0

    def tile(self):
        if self.count < self.MAX_TAGS:
            tag = f"s{self.count}"
            self.count += 1
        else:
            tag = self.free_tags.popleft()
        t = self.pool.tile([P, F], F32, tag=tag)
        return _Var(t, tag)

    def free(self, *tiles):
        for t in tiles:
            self.free_tags.append(t.tag)


def _build():
    nc = bacc.Bacc(None, target_bir_lowering=False)
    pred_h = nc.dram_tensor("pred", [NPER, 5], F32, kind="ExternalInput")
    tgt_h = nc.dram_tensor("target", [NPER, 5], F32, kind="ExternalInput")
    out_h = nc.dram_tensor("out", [P, 1], F32, kind="ExternalOutput")

    V = nc.vector
    G = nc.gpsimd
    A = nc.scalar

    with tile.TileContext(nc) as tc:
        with tc.tile_pool(name="io", bufs=1) as io_pool, \
             tc.tile_pool(name="consts", bufs=1) as cpool, \
             tc.tile_pool(name="main", bufs=1) as pool:
            s = _Slots(pool)

            _kcache = {}

            def konst(val):
                # [P,1] SBUF constant, for activation bias/scale operands
                if val not in _kcache:
                    t = cpool.tile([P, 1], F32, tag=f"c{len(_kcache)}")
                    nc.gpsimd.memset(t[:, :], val)
                    _kcache[val] = t
                return _kcache[val][:, :]

            PRED = io_pool.tile([P, F, 5], F32, tag="PRED")
            TGT = io_pool.tile([P, F, 5], F32, tag="TGT")
            nc.sync.dma_start(out=PRED, in_=pred_h[:, :].rearrange("(p f) c -> p f c", p=P))
            nc.sync.dma_start(out=TGT, in_=tgt_h[:, :].rearrange("(p f) c -> p f c", p=P))

            pcx, pcy, pw, ph, pth = (PRED[:, :, j] for j in range(5))
            tcx, tcy, tw, th, tth = (TGT[:, :, j] for j in range(5))

            def tt(eng, x, y, op):
                o = s.tile()
                eng.tensor_tensor(_ap(o), _ap(x), _ap(y), op)
                return o

            def ts(eng, x, scalar, op):
                o = s.tile()
                eng.tensor_scalar(_ap(o), _ap(x), scalar, None, op)
                return o

            def stt(x, scalar, y, op0, op1):
                # DVE only: (x op0 scalar) op1 y
                o = s.tile()
                V.scalar_tensor_tensor(_ap(o), _ap(x), scalar, _ap(y), op0, op1)
                return o

            def act(x, func, bias=0.0, scale=1.0, accum_out=None):
                o = s.tile()
                if isinstance(bias, float) and bias not in (0.0, 1.0):
                    bias = konst(bias)
                if isinstance(scale, float) and scale not in (0.0, 1.0):
                    scale = konst(scale)
                A.activation(out=_ap(o), in_=_ap(x),
                             func=func, bias=bias, scale=scale, accum_out=accum_out)
                return o

            # ---- Phase A: angles, sizes, deltas --------------------------------
            # ACT Sin is only accurate on ~(-pi, pi): cos(t) = sin(pi/2 - t)
            # works for t in [0, pi); cos(phi) for phi in (-pi, pi) uses the
            # half-angle identity 1 - 2*sin^2(phi/2).
            phi = tt(V, tth, pth, Op.subtract)
            cosp = act(pth, Fn.Sin, bias=HPI, scale=-1.0)
            sinp = act(pth, Fn.Sin)
            sinf = act(phi, Fn.Sin)
            sh = act(phi, Fn.Sin, scale=0.5)
            s.free(phi)
            shq = act(sh, Fn.Square)
            s.free(sh)
            cosf = s.tile()
            V.tensor_scalar(_ap(cosf), _ap(shq), -2.0, 1.0, Op.mult, Op.add)
            s.free(shq)
            a = act(pw, Fn.Copy, scale=0.5)
            b = act(ph, Fn.Copy, scale=0.5)
            a2 = act(tw, Fn.Copy, scale=0.5)
            b2 = act(th, Fn.Copy, scale=0.5)
            ar1 = tt(V, pw, ph, Op.mult)
            ar2 = tt(G, tw, th, Op.mult)
            Dx = tt(V, tcx, pcx, Op.subtract)
            Dy = tt(V, tcy, pcy, Op.subtract)

            t1 = tt(V, Dx, cosp, Op.mult)
            t2 = tt(V, Dy, sinp, Op.mult)
            dx = tt(V, t1, t2, Op.subtract)
            s.free(t1, t2)
            t3 = tt(G, Dx, sinp, Op.mult)
            t4 = tt(G, Dy, cosp, Op.mult)
            dy = tt(G, t3, t4, Op.add)
            s.free(t3, t4, Dx, Dy, cosp, sinp)

            ab = tt(V, a, b, Op.mult)
            ab2 = tt(G, a2, b2, Op.mult)

            # ---- Phase B: target geometry in pred frame ------------------------
            A_ = tt(V, a2, cosf, Op.mult)
            B_ = tt(V, b2, sinf, Op.mult)
            C_ = tt(G, a2, sinf, Op.mult)
            D_ = tt(G, b2, cosf, Op.mult)

            m1 = tt(V, dx, C_, Op.mult)
            m2 = tt(V, dy, A_, Op.mult)
            uu = tt(V, m1, m2, Op.add)
            s.free(m1, m2)
            m3 = tt(G, dx, D_, Op.mult)
            m4 = tt(G, dy, B_, Op.mult)
            vv = tt(G, m3, m4, Op.subtract)
            s.free(m3, m4)

            S1 = tt(V, A_, B_, Op.add)
            S2 = tt(V, A_, B_, Op.subtract)
            S3 = tt(G, C_, D_, Op.add)
            S4 = tt(G, C_, D_, Op.subtract)
            g0x = tt(V, dx, S1, Op.subtract)
            g2x = tt(V, dx, S1, Op.add)
            g1x = tt(V, dx, S2, Op.add)
            g3x = tt(V, dx, S2, Op.subtract)
            s.free(S1, S2)
            g0y = tt(G, dy, S4, Op.add)
            g2y = tt(G, dy, S4, Op.subtract)
            g1y = tt(G, dy, S3, Op.subtract)
            g3y = tt(G, dy, S3, Op.add)
            s.free(S3, S4)

            # ---- Phase C: pred corners in target frame -------------------------
            e1 = tt(V, a, dx, Op.subtract)
            e2 = stt(a, -1.0, dx, Op.mult, Op.subtract)     # -a - dx
            f1 = tt(G, b, dy, Op.subtract)
            f2 = stt(b, -1.0, dy, Op.mult, Op.subtract)     # -b - dy
            s.free(dx, dy)

            e1c = tt(V, e1, cosf, Op.mult)
            e2c = tt(V, e2, cosf, Op.mult)
            f1s = tt(V, f1, sinf, Op.mult)
            f2s = tt(V, f2, sinf, Op.mult)
            e1s = tt(G, e1, sinf, Op.mult)
            e2s = tt(G, e2, sinf, Op.mult)
            f1c = tt(G, f1, cosf, Op.mult)
            f2c = tt(G, f2, cosf, Op.mult)
            s.free(e1, e2, f1, f2, cosf, sinf)

            xi0 = tt(V, e2c, f2s, Op.subtract)
            xi1 = tt(V, e1c, f2s, Op.subtract)
            xi2 = tt(V, e1c, f1s, Op.subtract)
            xi3 = tt(V, e2c, f1s, Op.subtract)
            et0 = tt(G, e2s, f2c, Op.add)
            et1 = tt(G, e1s, f2c, Op.add)
            et2 = tt(G, e1s, f1c, Op.add)
            et3 = tt(G, e2s, f1c, Op.add)
            u1 = tt(V, e1c, e2c, Op.subtract)
            w1 = tt(V, f2s, f1s, Op.subtract)
            u2 = tt(G, e1s, e2s, Op.subtract)
            w2 = tt(G, f1c, f2c, Op.subtract)
            s.free(e1c, e2c, f1s, f2s, e1s, e2s, f1c, f2c)

            # ---- Phase D: safe reciprocals of the 8 edge slopes ----------------
            # inv(x) ~ x/(x^2+REPS): q = max(x*x, REPS) guards the approx
            # reciprocal (single custom-DVE op) against 0/denormal inputs.
            specs = [  # (x, post-scale, square/mul engine)
                (u1, 1.0, V), (u2, 1.0, G), (w1, 1.0, V), (w2, 1.0, G),
                (A_, 0.5, V), (C_, 0.5, G), (B_, 0.5, V), (D_, 0.5, G),
            ]
            invs = []
            for x, sc, eng in specs:
                q = tt(eng, x, x, Op.mult)
                qc = ts(V, q, 1e-30, Op.max)
                s.free(q)
                r = s.tile()
                V.reciprocal_approx_fast(out=_ap(r), in_=_ap(qc))
                s.free(qc)
                if sc == 1.0:
                    invs.append(tt(eng, x, r, Op.mult))
                else:
                    invs.append(stt(x, sc, r, Op.mult, Op.mult))
                s.free(r)
            iu1, iu2, iw1, iw2, iA, iC, iB, iD = invs
            s.free(u1, u2, w1, w2, A_, B_, C_, D_)

            # ---- Phase E: slab alphas |bound * inv| ----------------------------
            def alpha(eng, m, inv):
                t = tt(eng, m, inv, Op.mult)
                o = act(t, Fn.Abs)
                s.free(t)
                return o

            AXU = alpha(V, a2, iu1)
            AEU = alpha(G, b2, iu2)
            AXW = alpha(V, a2, iw1)
            AEW = alpha(G, b2, iw2)
            AXA = alpha(V, a, iA)
            AYC = alpha(G, b, iC)
            AXB = alpha(V, a, iB)
            AYD = alpha(G, b, iD)
            s.free(a, b, a2, b2)

            # ---- Phase F: per-edge clip intervals ------------------------------
            def edge(eng, c1, i1, s1, Aa1, c2, i2, s2, Aa2):
                # eng: engine for add/sub/mult ops (min ops + TS stay on DVE)
                b1 = tt(eng, c1, i1, Op.mult)
                b2_ = tt(eng, c2, i2, Op.mult)
                # n = -tlo ; h = thi   (sign depends on slope orientation)
                n1 = tt(eng, b1, Aa1, Op.add) if s1 > 0 else tt(eng, Aa1, b1, Op.subtract)
                h1 = tt(eng, Aa1, b1, Op.subtract) if s1 > 0 else tt(eng, Aa1, b1, Op.add)
                n2 = tt(eng, b2_, Aa2, Op.add) if s2 > 0 else tt(eng, Aa2, b2_, Op.subtract)
                h2 = tt(eng, Aa2, b2_, Op.subtract) if s2 > 0 else tt(eng, Aa2, b2_, Op.add)
                s.free(b1, b2_)
                nlo = tt(V, n1, n2, Op.min)      # = -lo
                hi = tt(V, h1, h2, Op.min)
                s.free(n1, n2, h1, h2)
                hi1 = ts(V, hi, 1.0, Op.min)
                rlo = act(nlo, Fn.Relu, scale=-1.0)  # relu(lo)
                s.free(nlo, hi)
                dte = tt(eng, hi1, rlo, Op.subtract)
                s.free(hi1, rlo)
                dt = act(dte, Fn.Relu)
                s.free(dte)
                return dt

            dtP0 = edge(V, xi0, iu1, +1, AXU, et0, iu2, +1, AEU)
            dtP1 = edge(G, xi1, iw1, +1, AXW, et1, iw2, +1, AEW)
            dtP2 = edge(G, xi2, iu1, -1, AXU, et2, iu2, -1, AEU)
            dtP3 = edge(V, xi3, iw1, -1, AXW, et3, iw2, -1, AEW)
            dtT0 = edge(G, g0x, iA, +1, AXA, g0y, iC, -1, AYC)
            dtT1 = edge(V, g1x, iB, +1, AXB, g1y, iD, +1, AYD)
            dtT2 = edge(G, g2x, iA, -1, AXA, g2y, iC, +1, AYC)
            dtT3 = edge(V, g3x, iB, -1, AXB, g3y, iD, -1, AYD)
            s.free(xi0, xi1, xi2, xi3, et0, et1, et2, et3,
                   g0x, g1x, g2x, g3x, g0y, g1y, g2y, g3y,
                   iu1, iu2, iw1, iw2, iA, iB, iC, iD,
                   AXU, AEU, AXW, AEW, AXA, AYC, AXB, AYD)

            # ---- Phase G: overlap, iou, loss -----------------------------------
            sp1 = tt(V, dtP0, dtP1, Op.add)
            sp2 = tt(G, dtP2, dtP3, Op.add)
            sp = tt(V, sp1, sp2, Op.add)
            ovP = tt(V, ab, sp, Op.mult)
            s.free(sp1, sp2, sp, ab, dtP0, dtP1, dtP2, dtP3)

            x0 = tt(V, ab2, uu, Op.subtract)
            c0 = tt(V, dtT0, x0, Op.mult)
            x2 = tt(V, ab2, uu, Op.add)
            c2_ = tt(V, dtT2, x2, Op.mult)
            x1 = tt(G, ab2, vv, Op.add)
            c1_ = tt(G, dtT1, x1, Op.mult)
            x3 = tt(G, ab2, vv, Op.subtract)
            c3_ = tt(G, dtT3, x3, Op.mult)
            s.free(x0, x1, x2, x3, uu, vv, ab2, dtT0, dtT1, dtT2, dtT3)
            cs1 = tt(V, c0, c2_, Op.add)
            cs2 = tt(G, c1_, c3_, Op.add)
            ovT = tt(V, cs1, cs2, Op.add)
            ov = tt(V, ovP, ovT, Op.add)
            s.free(c0, c1_, c2_, c3_, cs1, cs2, ovP, ovT)

            # log(iou) = max(ln(ov) - ln(den), ln(EPS)) -- no division needed.
            s12 = tt(G, ar1, ar2, Op.add)
            den = tt(V, s12, ov, Op.subtract)
            s.free(ar1, ar2, s12)
            ovc = ts(V, ov, 1e-35, Op.max)          # guard ln() against ov <= 0
            s.free(ov)
            lnd = act(den, Fn.Ln, bias=EPS)
            s.free(den)
            lno = act(ovc, Fn.Ln)
            s.free(ovc)
            df = tt(V, lno, lnd, Op.subtract)
            s.free(lno, lnd)

            acc = pool.tile([P, 1], F32, tag="acc")
            lg = s.tile()
            V.tensor_scalar(_ap(lg), _ap(df), float(np.log(EPS)), 0.0, Op.max,
                            Op.add, accum_out=acc[:, 0:1])
            s.free(df, lg)

            nc.sync.dma_start(out=out_h[:, :], in_=acc[:, :])

    nc.compile()
    return nc


_NC = None


def _get_nc():
    global _NC
    if _NC is None:
        _NC = _build()
    return _NC


class _Runner:
    """Cached PJRT executor for the compiled Bass module: same lowering path
    as bass_utils.run_bass_kernel_spmd (bass2jax custom call + shard_map over
    8 cores), but the jitted callable is built once and reused, so repeat
    kernel() calls skip retracing/re-compiling."""

    def __init__(self, nc):
        import jax
        from jax.sharding import Mesh, PartitionSpec
        try:
            from jax.experimental.shard_map import shard_map
        except ImportError:
            from jax.shard_map import shard_map  # newer jax
        from concourse import bass2jax, mybir as mb

        bass2jax.install_neuronx_cc_hook()
        self.jax = jax
        partition_name = (nc.partition_id_tensor.name
                          if nc.partition_id_tensor else None)
        in_names, out_names, out_avals, zero_outs = [], [], [], []
        for alloc in nc.m.functions[0].allocations:
            if not isinstance(alloc, mb.MemoryLocationSet):
                continue
            name = alloc.memorylocations[0].name
            if alloc.kind == "ExternalInput":
                if name != partition_name:
                    in_names.append(name)
            elif alloc.kind == "ExternalOutput":
                shape = tuple(alloc.tensor_shape)
                dtype = mb.dt.np(alloc.dtype)
                out_names.append(name)
                out_avals.append(jax.core.ShapedArray(shape, dtype))
                zero_outs.append(np.zeros((NCORES * shape[0],) + shape[1:], dtype))
        self.in_names = list(in_names)
        self.out_names = list(out_names)
        self.zero_outs = zero_outs
        n_params = len(in_names)
        all_names = in_names + out_names
        if partition_name is not None:
            all_names = all_names + [partition_name]

        def _body(*args):
            operands = list(args)
            if partition_name is not None:
                operands.append(bass2jax.partition_id_tensor())
            outs = bass2jax._bass_exec_p.bind(
                *operands,
                out_avals=tuple(out_avals),
                in_names=tuple(all_names),
                out_names=tuple(out_names),
                lowering_input_output_aliases=(),
                sim_require_finite=True,
                sim_require_nnan=True,
                nc=nc,
            )
            return tuple(outs)

        devices = jax.devices()[:NCORES]
        mesh = Mesh(np.asarray(devices), ("core",))
        n_outs = len(out_names)
        self.fn = jax.jit(
            shard_map(_body, mesh=mesh,
                      in_specs=(PartitionSpec("core"),) * (n_params + n_outs),
                      out_specs=(PartitionSpec("core"),) * n_outs,
                      check_rep=False),
            donate_argnums=tuple(range(n_params, n_params + n_outs)),
            keep_unused=True,
        )

    def __call__(self, pred, target):
        ins = {"pred": pred, "target": target}
        args = [ins[n] for n in self.in_names] + [z.copy() for z in self.zero_outs]
        outs = self.fn(*args)
        return [np.asarray(o) for o in outs]


_RUNNER = None


def _get_runner():
    global _RUNNER
    if _RUNNER is None:
        _RUNNER = _Runner(_get_nc())
    return _RUNNER


def kernel(pred: np.ndarray, target: np.ndarray) -> np.ndarray:
    pred = np.ascontiguousarray(np.asarray(pred, dtype=np.float32))
    target = np.ascontiguousarray(np.asarray(target, dtype=np.float32))
    assert pred.shape == (N_TOTAL, 5) and target.shape == (N_TOTAL, 5)

    runner = _get_runner()
    outs = runner(pred, target)   # out partials, concatenated over cores
    total = outs[0].astype(np.float64).sum()
    return np.float32(-(total / N_TOTAL))
